# revision 13
# baseline (speedup 1.0000x reference)
"""Trainium2 Bass kernel for a circular-padded 3x3 conv cellular-automaton step.

Computation (per image):
    z   = conv3x3_circular(x, Wc) ;  Wc = w1 @ w_perc  (host-fused, [96,12,3,3])
    h   = relu(z + b1)
    u   = w2 @ h + b2
    out = x + (mask > 0.5) * u

Mapping (per core, B=16 split 8 ways -> 2 images/core):
  * conv as ONE matmul per image row: K=108 partitions (dj,di,c), all nine
    (dj,di) row/col-shifted copies materialized straight from DRAM by a
    single 108-partition DMA per 16-row chunk (host stores a di-replicated
    padded copy so the AP stays 3-dim).  Weights bf16, fp32 PSUM.
  * relu+bias split: ScalarE 3 rows, VectorE 1 row per 4-row supertile,
    output bf16 `h` with a ones-row (row 96) so b2 rides in matmul 2.
  * second matmul (w2 [12,96] zero-padded to [97,32], b2 in row 96) as 4
    column-group matmuls -> one [128, 384] PSUM tile per supertile
    (partition 32j+c = row j, channel c).
  * mask multiply on VectorE against a host-prelaid [128, 96*384] bf16 mask;
    +x on GPSIMD from a (j,c)-planar bf16 copy of x; bf16 output staged per
    chunk and stored with one 48-partition DMA; host inverse-permutes.
  * post-pass on the emitted IR removes back-to-back LDWEIGHTS reloads of
    identical weights and hoists the 4 col-group LDWEIGHTS of matmul 2 ahead
    of its matmuls, so the PE streams matmuls back-to-back.
"""

import os
import sys

if "/opt/trn_rl_repo" not in sys.path:
    sys.path.insert(0, "/opt/trn_rl_repo")

_NO_PASSES = os.environ.get("K_NO_PASSES", "0") == "1"
_NO_STAMPS = os.environ.get("K_NO_STAMPS", "0") == "1"

from contextlib import ExitStack

import numpy as np
import ml_dtypes

import concourse.bass as bass
import concourse.tile as tile
from concourse import mybir
from concourse.bass_utils import run_bass_kernel_spmd

B, C, H, W = 16, 12, 384, 384
CH = 96                      # hidden channels
NCORES = 8
BLOC = B // NCORES           # images per core
W2 = W + 2                   # circular-padded row length
PADH = H + 4                 # padded rows: 1 top + 3 bottom
CHUNK = 16                   # image rows per processing chunk
ST = 4                       # rows per supertile (one per PE column group)
NCHUNK = H // CHUNK
NST = CHUNK // ST            # supertiles per chunk
XQLEN = 15 * W2 + W          # loaded free length per chunk (row 15 + conv window)
MTILES = H // ST             # 96 supertile row-groups per image
STW = NST * W                # supertile-layout free length per chunk
R4 = H // ST                 # 96 4-row groups per image

_BF16 = mybir.dt.bfloat16
_F32 = mybir.dt.float32


def _spill_waits(nc):
    """walrus/trn2 here accepts at most ONE sync-wait per instruction; move
    excess waits onto NoOps inserted immediately before, on the same engine."""
    nspill = 0
    for bbwrap in list(nc.bb_map.values()):
        bb = bbwrap.bb
        out = []
        for inst in bb.instructions:
            si = inst.sync_info
            if si is not None and si.on_wait and len(si.on_wait) > 1:
                waits = list(si.on_wait)
                for w in waits[1:]:
                    nop = mybir.InstNoOp(
                        name=nc.get_next_instruction_name(),
                        engine=inst.engine,
                        sync_info=mybir.SyncInfo(on_wait=[w], on_update=[]),
                        bass_nofuse=True,
                    )
                    nc.register_instruction(nop)
                    out.append(nop)
                    nspill += 1
                si.on_wait = waits[:1]
            out.append(inst)
        try:
            bb.instructions = out
        except Exception:
            bb.instructions.clear()
            bb.instructions.extend(out)
    return nspill


def _ldw_sig(inst):
    """Signature of an InstLdweights: which weights it loads where."""
    ap = inst.ins[0]
    try:
        tname = ap.tensor_name
    except Exception:
        tname = getattr(ap, "name", str(ap))
    return (str(tname), str(getattr(ap, "offset", "")), str(ap.ap), str(inst.tile_position))


def _ldw_cols(inst):
    """(col_lo, col_hi) range of PE column groups this LDWEIGHTS covers."""
    tp = inst.tile_position or (0, 0)
    ts = inst.tile_size
    ncols = ts[1] if ts else 128
    return (tp[1], tp[1] + ncols)


def _merge_waits(dst_inst, waits):
    si = dst_inst.sync_info
    if si is None:
        dst_inst.sync_info = mybir.SyncInfo(on_wait=list(waits), on_update=[])
    else:
        si.on_wait = list(si.on_wait) + list(waits)


def _dedup_hoist_ldweights(nc):
    """1) Hoist each run of interleaved [LDW, MM]*k (same weights tensor,
    distinct col groups) into [LDW*k, MM*k] so the matmuls issue
    back-to-back and overlap across PE column groups.
    2) Remove LDWEIGHTS whose covered column groups already hold exactly
    those weights.  Waits on removed LDWs move to the next tensor inst."""
    n_removed = 0
    n_hoisted = 0
    for bbwrap in list(nc.bb_map.values()):
        bb = bbwrap.bb
        insts = list(bb.instructions)
        # --- pass 1: hoist LDWs in alternating LDW/MM runs -------------
        tpos = [
            i
            for i, inst in enumerate(insts)
            if isinstance(inst, (mybir.InstLdweights, mybir.InstMatmult))
        ]
        i = 0
        while i < len(tpos):
            # find run start: LDW at tpos[i]
            if not isinstance(insts[tpos[i]], mybir.InstLdweights):
                i += 1
                continue
            # collect alternating LDW, MM pairs
            j = i
            pairs = []
            while (
                j + 1 < len(tpos)
                and isinstance(insts[tpos[j]], mybir.InstLdweights)
                and isinstance(insts[tpos[j + 1]], mybir.InstMatmult)
            ):
                pairs.append((tpos[j], tpos[j + 1]))
                j += 2
            if len(pairs) >= 2:
                # split the run into consecutive same-weights-tensor segments
                sigs = [_ldw_sig(insts[a])[0] for a, _ in pairs]
                seg_start = 0
                for k in range(1, len(pairs) + 1):
                    if k == len(pairs) or sigs[k] != sigs[seg_start]:
                        seg = pairs[seg_start:k]
                        cols = [insts[a].tile_position for a, _ in seg]
                        colset = {c[1] for c in cols if c}
                        if len(seg) >= 2 and len(colset) == len(seg):
                            # distinct col groups -> hoist LDWs before MMs
                            slots = [s for pr in seg for s in pr]
                            newseq = [insts[a] for a, _ in seg] + [
                                insts[b] for _, b in seg
                            ]
                            for slot, inst in zip(slots, newseq):
                                insts[slot] = inst
                            n_hoisted += 1
                        seg_start = k
            i = max(j, i + 1)
        # --- pass 2: dedup identical reloads ---------------------------
        loaded = {}  # col_group (0,32,64,96) -> sig
        out = []
        pending_waits = []
        for inst in insts:
            if isinstance(inst, mybir.InstLdweights):
                sig = _ldw_sig(inst)
                lo, hi = _ldw_cols(inst)
                groups = [g for g in (0, 32, 64, 96) if lo <= g < hi]
                if groups and all(loaded.get(g) == sig for g in groups):
                    si = inst.sync_info
                    if si is not None and si.on_wait:
                        pending_waits.extend(si.on_wait)
                    if si is not None and si.on_update:
                        # updates must fire: keep the instruction instead
                        for g in groups:
                            loaded[g] = sig
                        out.append(inst)
                        continue
                    n_removed += 1
                    continue
                for g in groups:
                    loaded[g] = sig
                out.append(inst)
            else:
                if pending_waits and inst.engine == mybir.EngineType.PE:
                    _merge_waits(inst, pending_waits)
                    pending_waits = []
                out.append(inst)
        if pending_waits:
            _merge_waits(out[-1], pending_waits)
        try:
            bb.instructions = out
        except Exception:
            bb.instructions.clear()
            bb.instructions.extend(out)
    return n_removed, n_hoisted


def _build_nc(reps=1):
    nc = bass.Bass()

    xrep = nc.declare_dram_parameter("xrep", [BLOC, 3, C, PADH, W2], _BF16, isOutput=False)
    xjc = nc.declare_dram_parameter("xjc", [BLOC, NCHUNK, 108, STW], _BF16, isOutput=False)
    wa = nc.declare_dram_parameter("wa", [108, CH], _BF16, isOutput=False)
    w2p = nc.declare_dram_parameter("w2p", [CH + 1, 32], _BF16, isOutput=False)
    b1 = nc.declare_dram_parameter("b1", [CH, 1], _F32, isOutput=False)
    m128 = nc.declare_dram_parameter("m128", [108, MTILES * W], _BF16, isOutput=False)
    out = nc.declare_dram_parameter("out", [BLOC, NCHUNK, 108, STW], _BF16, isOutput=True)
    dbg_xq = nc.declare_dram_parameter("dbg_xq", [108, XQLEN], _BF16, isOutput=True)
    dbg_ht = nc.declare_dram_parameter("dbg_ht", [CH + 1, ST, W], _BF16, isOutput=True)
    dbg_xt = nc.declare_dram_parameter("dbg_xt", [128, ST, W], _BF16, isOutput=True)

    with tile.TileContext(nc) as tc, ExitStack() as ctx:
        state = _setup(ctx, tc, wa, w2p, b1, m128)
        state["dbg"] = (dbg_xq, dbg_ht, dbg_xt)
        if reps == 1:
            _loop_body(tc, state, xrep, xjc, out)
        else:
            with tc.For_i(0, reps, 1):
                _loop_body(tc, state, xrep, xjc, out)
    if not _NO_PASSES:
        _dedup_hoist_ldweights(nc)
    _spill_waits(nc)
    return nc


def _setup(ctx, tc, wa, w2p, b1, m128):
    nc = tc.nc

    const = ctx.enter_context(tc.tile_pool(name="const", bufs=1))
    ump = ctx.enter_context(tc.tile_pool(name="um", bufs=2))
    zp = ctx.enter_context(tc.tile_pool(name="z", bufs=3, space="PSUM"))
    up = ctx.enter_context(tc.tile_pool(name="u", bufs=2, space="PSUM"))

    wa_sb = const.tile([108, CH], _BF16)
    nc.sync.dma_start(out=wa_sb, in_=wa[:, :])
    w2p_sb = const.tile([CH + 1, 32], _BF16)
    nc.sync.dma_start(out=w2p_sb, in_=w2p[:, :])
    b1_sb = const.tile([CH, 1], _F32)
    nc.sync.dma_start(out=b1_sb, in_=b1[:, :])
    m128_sb = const.tile([128, MTILES * W], _BF16)
    nc.vector.memset(m128_sb[96:128, :], 0.0)
    nc.sync.dma_start(out=m128_sb[0:108, :], in_=m128[:, :])

    # manually double-buffered tiles (stable addresses):
    #  - ht: constant ones-row (row 96) carries b2 through the second matmul
    #  - xq: conv input, partitions (dj,di,c), straight from DRAM
    #  - xt: x in supertile layout (partitions 32j+c), holey 48-partition DMA
    #  - ot: output staging in supertile layout
    hts = [
        const.tile([CH + 1, ST, W], _BF16, name=f"ht{i}", tag=f"ht{i}")
        for i in range(3)
    ]
    xqs = [
        const.tile([108, XQLEN], _BF16, name=f"xqt{i}", tag=f"xqt{i}") for i in range(2)
    ]
    xts = [
        const.tile([128, ST, W], _BF16, name=f"xtt{i}", tag=f"xtt{i}") for i in range(2)
    ]
    ots = [
        const.tile([128, ST, W], _BF16, name=f"ott{i}", tag=f"ott{i}") for i in range(2)
    ]
    for t in hts:
        nc.vector.memset(t, 0.0)
        nc.vector.memset(t[CH : CH + 1, :, :], 1.0)
    for t in xqs + xts + ots:
        nc.vector.memset(t, 0.0)

    # warmup matmuls: absorb the weight-load DMA waits on the PE clock
    zw = zp.tile([CH, 2, 512], _F32, tag="z2")
    nc.tensor.matmul(zw[:, 0, 0:1], wa_sb, xqs[0][:, 0:1], start=True, stop=True)
    uw = up.tile([128, 512], _F32, tag="u")
    nc.tensor.matmul(
        uw[0:32, 0:1], w2p_sb, hts[0][:, 0, 0:1], start=True, stop=True,
        tile_position=(0, 0),
    )

    return dict(
        ump=ump, zp=zp, up=up,
        wa_sb=wa_sb, w2p_sb=w2p_sb, b1_sb=b1_sb, m128_sb=m128_sb,
        hts=hts, xqs=xqs, xts=xts, ots=ots,
    )


def _loop_body(tc, state, xrep, xjc, out):
    nc = tc.nc
    add = mybir.AluOpType.add
    mult = mybir.AluOpType.mult
    amax = mybir.AluOpType.max
    relu = mybir.ActivationFunctionType.Relu
    ump, zp, up = state["ump"], state["zp"], state["up"]
    wa_sb, w2p_sb, b1_sb, m128_sb = (
        state["wa_sb"], state["w2p_sb"], state["b1_sb"], state["m128_sb"],
    )
    hts, xqs, xts, ots = state["hts"], state["xqs"], state["xts"], state["ots"]

    nbuf = 0
    ncbuf = 0
    for b in range(BLOC):
        for chk in range(NCHUNK):
            r0 = chk * CHUNK
            xq = xqs[ncbuf % 2]
            xt = xts[ncbuf % 2]
            ot = ots[ncbuf % 2]
            ncbuf += 1

            # one DMA: partitions (di*12+c)*3 + dj <- row-and-column-shifted
            # copies of x rows r0..r0+15 (wrap-padded), straight from DRAM.
            # (di,c) outermost so the descriptor splitter spreads the 36
            # partition groups across all 16 SDMA engines, not 3.
            src = bass.AP(
                tensor=xrep,
                offset=(b * 3 * C * PADH + r0) * W2,
                ap=[[PADH * W2, 3 * C], [1, 3], [1, XQLEN]],
            )
            nc.sync.dma_start(out=xq, in_=src)
            # x in supertile layout: partitions 32j+c (pads host-zeroed)
            nc.sync.dma_start(out=xt[0:108, :, :], in_=xjc[b, chk, :, :])

            for st in range(NST):
                tglob = chk * NST + st
                tsim = (b * NCHUNK + chk) * NST + st
                z2a = zp.tile([CH, 2, 512], _F32, tag="z2")
                z2b = zp.tile([CH, 2, 512], _F32, tag="z2")
                # manual sim-time stamps group the conv and mm2 waves so the
                # PE stream stays weight-stationary within each wave; mm2 of
                # supertile t lands between conv(t+1) and conv(t+2)
                with tc.tile_wait_until(4 * tsim, enable=not _NO_STAMPS):
                    for j in range(ST):
                        q = st * ST + j
                        zt = (z2a if j < 2 else z2b)[:, j % 2, 0:W]
                        nc.tensor.matmul(
                            zt,
                            wa_sb,
                            xq[0:108, q * W2 : q * W2 + W],
                            start=True,
                            stop=True,
                        )

                ht = hts[nbuf % 3]
                nc.scalar.activation(
                    out=ht[0:CH, 0:2, :], in_=z2a[:, :, 0:W], func=relu, bias=b1_sb
                )
                nc.scalar.activation(
                    out=ht[0:CH, 2:3, :], in_=z2b[:, 0:1, 0:W], func=relu, bias=b1_sb
                )
                nc.vector.tensor_scalar(
                    ht[0:CH, 3:4, :], z2b[:, 1:2, 0:W], b1_sb, 0.0, add, amax
                )

                u = up.tile([128, 512], _F32, tag="u")
                with tc.tile_wait_until(4 * tsim + 10, enable=not _NO_STAMPS):
                    for j in range(ST):
                        nc.tensor.matmul(
                            u[32 * j : 32 * j + 32, 0:W],
                            w2p_sb,
                            ht[:, j, :],
                            start=True,
                            stop=True,
                            tile_position=(0, 32 * j),
                        )

                if b == 0 and chk == 0 and st == 0 and "dbg" in state:
                    dxq, dht, dxt = state["dbg"]
                    nc.sync.dma_start(out=dxq[:, :], in_=xq)
                    nc.sync.dma_start(out=dht[:, :, :], in_=ht)
                    nc.sync.dma_start(out=dxt[:, :, :], in_=xt)
                um = ump.tile([128, W], _BF16)
                nc.vector.tensor_tensor(
                    um, u[:, 0:W], m128_sb[:, tglob * W : tglob * W + W], mult
                )
                nc.gpsimd.tensor_tensor(
                    ot[:, st, :], um, xt[:, st, :], add
                )
                nbuf += 1

            nc.sync.dma_start(out=out[b, chk, :, :], in_=ot[0:108, :, :])


_NC_CACHE = {}


def _get_nc():
    if "nc" not in _NC_CACHE:
        _NC_CACHE["nc"] = _build_nc()
    return _NC_CACHE["nc"]


def _prep_inputs(x, w_perc, w1, b1, w2, b2, mask):
    bf16 = ml_dtypes.bfloat16
    wc = np.einsum("hp,pcij->hcij", w1, w_perc).astype(np.float32)  # [96,12,3,3]
    # wa[(di*12 + c)*3 + dj, h] = wc[h, c, di, dj]
    wdicdj = wc.transpose(2, 1, 3, 0)  # [di, c, dj, h]
    wa = np.ascontiguousarray(wdicdj.reshape(108, CH)).astype(bf16)
    w2pv = np.zeros((CH + 1, 32), np.float32)
    w2pv[0:CH, 0:C] = w2.T
    w2pv[CH, 0:C] = b2
    w2pv = w2pv.astype(bf16)
    b1c = np.ascontiguousarray(b1.reshape(CH, 1)).astype(np.float32)

    mbit = (mask > 0.5).astype(np.float32)
    m128 = np.zeros((108, MTILES * W), np.float32)
    for j in range(ST):
        rows = mbit[j::ST, :].reshape(MTILES * W)
        for c in range(C):
            m128[32 * j + c] = rows
    m128 = m128.astype(bf16)

    xb16 = x.astype(bf16)
    in_maps = []
    for core in range(NCORES):
        xs = xb16[core * BLOC : (core + 1) * BLOC]
        # di-replicated wrap-padded copy: xrep[b, di, c, r, w] = xpadbig[b, c, r+di, w]
        xpadbig = np.pad(xs, ((0, 0), (0, 0), (1, 5), (1, 1)), mode="wrap")
        xrep = np.stack(
            [xpadbig[:, :, di : di + PADH, :] for di in range(3)], axis=1
        )  # [BLOC, 3, C, PADH, W2]
        # supertile layout: xjc[b, chk, 32*j+c, s*W+w] = x[b, c, 16*chk+4*s+j, w]
        xs = xb16[core * BLOC : (core + 1) * BLOC].astype(np.float32)
        xst = np.zeros((BLOC, NCHUNK, ST, 32, NST, W), np.float32)
        xst[:, :, :, 0:C] = xs.reshape(BLOC, C, NCHUNK, NST, ST, W).transpose(
            0, 2, 4, 1, 3, 5
        )
        xjc = np.ascontiguousarray(
            xst.reshape(BLOC, NCHUNK, 128, STW)[:, :, 0:108].astype(ml_dtypes.bfloat16)
        )
        in_maps.append(
            {
                "xrep": np.ascontiguousarray(xrep),
                "xjc": xjc,
                "wa": wa,
                "w2p": w2pv,
                "b1": b1c,
                "m128": m128,
            }
        )
    return in_maps


def _unshard_out(core_outs):
    full = np.empty((B, C, H, W), np.float32)
    for core, o in enumerate(core_outs):
        o = np.asarray(o).astype(np.float32)
        o128 = np.zeros((BLOC, NCHUNK, 128, STW), np.float32)
        o128[:, :, 0:108] = o.reshape(BLOC, NCHUNK, 108, STW)
        o = o128.reshape(BLOC, NCHUNK, ST, 32, NST, W)[:, :, :, 0:C]
        # [b, chk, j, c, s, w] -> [b, c, (chk s j), w]
        o = o.transpose(0, 3, 1, 4, 2, 5).reshape(BLOC, C, H, W)
        full[core * BLOC : (core + 1) * BLOC] = o
    return full


def kernel(x, w_perc, w1, b1, w2, b2, mask):
    x = np.asarray(x, dtype=np.float32)
    in_maps = _prep_inputs(
        x,
        np.asarray(w_perc, np.float32),
        np.asarray(w1, np.float32),
        np.asarray(b1, np.float32),
        np.asarray(w2, np.float32),
        np.asarray(b2, np.float32),
        np.asarray(mask, np.float32),
    )
    nc = _get_nc()
    res = run_bass_kernel_spmd(nc, in_maps, core_ids=list(range(NCORES)))
    return _unshard_out([r["out"] for r in res.results])


# revision 14
# speedup vs baseline: 1.1629x; 1.1629x over previous
"""Trainium2 Bass kernel for a circular-padded 3x3 conv cellular-automaton step.

Computation (per image):
    z   = conv3x3_circular(x, Wc) ;  Wc = w1 @ w_perc  (host-fused, [96,12,3,3])
    h   = relu(z + b1)
    u   = w2 @ h + b2
    out = x + (mask > 0.5) * u

Mapping (per core, B=16 split 8 ways -> 2 images/core):
  * conv as ONE matmul per image row: K=108 partitions (dj,di,c), all nine
    (dj,di) row/col-shifted copies materialized straight from DRAM by a
    single 108-partition DMA per 16-row chunk (host stores a di-replicated
    padded copy so the AP stays 3-dim).  Weights bf16, fp32 PSUM.
  * relu+bias split: ScalarE 3 rows, VectorE 1 row per 4-row supertile,
    output bf16 `h` with a ones-row (row 96) so b2 rides in matmul 2.
  * second matmul (w2 [12,96] zero-padded to [97,32], b2 in row 96) as 4
    column-group matmuls -> one [128, 384] PSUM tile per supertile
    (partition 32j+c = row j, channel c).
  * mask multiply on VectorE against a host-prelaid [128, 96*384] bf16 mask;
    +x on GPSIMD from a (j,c)-planar bf16 copy of x; bf16 output staged per
    chunk and stored with one 48-partition DMA; host inverse-permutes.
  * post-pass on the emitted IR removes back-to-back LDWEIGHTS reloads of
    identical weights and hoists the 4 col-group LDWEIGHTS of matmul 2 ahead
    of its matmuls, so the PE streams matmuls back-to-back.
"""

import os
import sys

if "/opt/trn_rl_repo" not in sys.path:
    sys.path.insert(0, "/opt/trn_rl_repo")

_NO_PASSES = os.environ.get("K_NO_PASSES", "0") == "1"
_NO_STAMPS = os.environ.get("K_NO_STAMPS", "0") == "1"

from contextlib import ExitStack

import numpy as np
import ml_dtypes

import concourse.bass as bass
import concourse.tile as tile
from concourse import mybir
from concourse.bass_utils import run_bass_kernel_spmd

B, C, H, W = 16, 12, 384, 384
CH = 96                      # hidden channels
NCORES = 8
BLOC = B // NCORES           # images per core
W2 = W + 2                   # circular-padded row length
PADH = H + 4                 # padded rows: 1 top + 3 bottom
CHUNK = 16                   # image rows per processing chunk
ST = 4                       # rows per supertile (one per PE column group)
NCHUNK = H // CHUNK
NST = CHUNK // ST            # supertiles per chunk
XQLEN = 15 * W2 + W          # loaded free length per chunk (row 15 + conv window)
MTILES = H // ST             # 96 supertile row-groups per image
STW = NST * W                # supertile-layout free length per chunk
R4 = H // ST                 # 96 4-row groups per image

_BF16 = mybir.dt.bfloat16
_F32 = mybir.dt.float32


def _spill_waits(nc):
    """walrus/trn2 here accepts at most ONE sync-wait per instruction; move
    excess waits onto NoOps inserted immediately before, on the same engine."""
    nspill = 0
    for bbwrap in list(nc.bb_map.values()):
        bb = bbwrap.bb
        out = []
        for inst in bb.instructions:
            si = inst.sync_info
            if si is not None and si.on_wait and len(si.on_wait) > 1:
                waits = list(si.on_wait)
                for w in waits[1:]:
                    nop = mybir.InstNoOp(
                        name=nc.get_next_instruction_name(),
                        engine=inst.engine,
                        sync_info=mybir.SyncInfo(on_wait=[w], on_update=[]),
                        bass_nofuse=True,
                    )
                    nc.register_instruction(nop)
                    out.append(nop)
                    nspill += 1
                si.on_wait = waits[:1]
            out.append(inst)
        try:
            bb.instructions = out
        except Exception:
            bb.instructions.clear()
            bb.instructions.extend(out)
    return nspill


def _ldw_sig(inst):
    """Signature of an InstLdweights: which weights it loads where."""
    ap = inst.ins[0]
    try:
        tname = ap.tensor_name
    except Exception:
        tname = getattr(ap, "name", str(ap))
    return (str(tname), str(getattr(ap, "offset", "")), str(ap.ap), str(inst.tile_position))


def _ldw_cols(inst):
    """(col_lo, col_hi) range of PE column groups this LDWEIGHTS covers."""
    tp = inst.tile_position or (0, 0)
    ts = inst.tile_size
    ncols = ts[1] if ts else 128
    return (tp[1], tp[1] + ncols)


def _merge_waits(dst_inst, waits):
    si = dst_inst.sync_info
    if si is None:
        dst_inst.sync_info = mybir.SyncInfo(on_wait=list(waits), on_update=[])
    else:
        si.on_wait = list(si.on_wait) + list(waits)


def _dedup_hoist_ldweights(nc):
    """1) Hoist each run of interleaved [LDW, MM]*k (same weights tensor,
    distinct col groups) into [LDW*k, MM*k] so the matmuls issue
    back-to-back and overlap across PE column groups.
    2) Remove LDWEIGHTS whose covered column groups already hold exactly
    those weights.  Waits on removed LDWs move to the next tensor inst."""
    n_removed = 0
    n_hoisted = 0
    for bbwrap in list(nc.bb_map.values()):
        bb = bbwrap.bb
        insts = list(bb.instructions)
        # --- pass 1: hoist LDWs in alternating LDW/MM runs -------------
        tpos = [
            i
            for i, inst in enumerate(insts)
            if isinstance(inst, (mybir.InstLdweights, mybir.InstMatmult))
        ]
        i = 0
        while i < len(tpos):
            # find run start: LDW at tpos[i]
            if not isinstance(insts[tpos[i]], mybir.InstLdweights):
                i += 1
                continue
            # collect alternating LDW, MM pairs
            j = i
            pairs = []
            while (
                j + 1 < len(tpos)
                and isinstance(insts[tpos[j]], mybir.InstLdweights)
                and isinstance(insts[tpos[j + 1]], mybir.InstMatmult)
            ):
                pairs.append((tpos[j], tpos[j + 1]))
                j += 2
            if len(pairs) >= 2:
                # split the run into consecutive same-weights-tensor segments
                sigs = [_ldw_sig(insts[a])[0] for a, _ in pairs]
                seg_start = 0
                for k in range(1, len(pairs) + 1):
                    if k == len(pairs) or sigs[k] != sigs[seg_start]:
                        seg = pairs[seg_start:k]
                        cols = [insts[a].tile_position for a, _ in seg]
                        colset = {c[1] for c in cols if c}
                        if len(seg) >= 2 and len(colset) == len(seg):
                            # distinct col groups -> hoist LDWs before MMs
                            slots = [s for pr in seg for s in pr]
                            newseq = [insts[a] for a, _ in seg] + [
                                insts[b] for _, b in seg
                            ]
                            for slot, inst in zip(slots, newseq):
                                insts[slot] = inst
                            n_hoisted += 1
                        seg_start = k
            i = max(j, i + 1)
        # --- pass 2: dedup identical reloads ---------------------------
        loaded = {}  # col_group (0,32,64,96) -> sig
        out = []
        pending_waits = []
        for inst in insts:
            if isinstance(inst, mybir.InstLdweights):
                sig = _ldw_sig(inst)
                lo, hi = _ldw_cols(inst)
                groups = [g for g in (0, 32, 64, 96) if lo <= g < hi]
                if groups and all(loaded.get(g) == sig for g in groups):
                    si = inst.sync_info
                    if si is not None and si.on_wait:
                        pending_waits.extend(si.on_wait)
                    if si is not None and si.on_update:
                        # updates must fire: keep the instruction instead
                        for g in groups:
                            loaded[g] = sig
                        out.append(inst)
                        continue
                    n_removed += 1
                    continue
                for g in groups:
                    loaded[g] = sig
                out.append(inst)
            else:
                if pending_waits and inst.engine == mybir.EngineType.PE:
                    _merge_waits(inst, pending_waits)
                    pending_waits = []
                out.append(inst)
        if pending_waits:
            _merge_waits(out[-1], pending_waits)
        try:
            bb.instructions = out
        except Exception:
            bb.instructions.clear()
            bb.instructions.extend(out)
    return n_removed, n_hoisted


def _build_nc(reps=1):
    nc = bass.Bass()

    xrep = nc.declare_dram_parameter("xrep", [BLOC, 3, C, PADH, W2], _BF16, isOutput=False)
    xjc = nc.declare_dram_parameter("xjc", [BLOC, NCHUNK, 108, STW], _BF16, isOutput=False)
    wa = nc.declare_dram_parameter("wa", [108, CH], _BF16, isOutput=False)
    w2p = nc.declare_dram_parameter("w2p", [CH + 1, 32], _BF16, isOutput=False)
    b1 = nc.declare_dram_parameter("b1", [CH, 1], _F32, isOutput=False)
    m128 = nc.declare_dram_parameter("m128", [108, MTILES * W], _BF16, isOutput=False)
    out = nc.declare_dram_parameter("out", [BLOC, NCHUNK, 108, STW], _BF16, isOutput=True)
    dbg_xq = nc.declare_dram_parameter("dbg_xq", [108, XQLEN], _BF16, isOutput=True)
    dbg_ht = nc.declare_dram_parameter("dbg_ht", [CH + 1, ST, W], _BF16, isOutput=True)
    dbg_xt = nc.declare_dram_parameter("dbg_xt", [128, ST, W], _BF16, isOutput=True)

    with tile.TileContext(nc) as tc, ExitStack() as ctx:
        state = _setup(ctx, tc, wa, w2p, b1, m128)
        state["dbg"] = (dbg_xq, dbg_ht, dbg_xt)
        if reps == 1:
            _loop_body(tc, state, xrep, xjc, out)
        else:
            with tc.For_i(0, reps, 1):
                _loop_body(tc, state, xrep, xjc, out)
    if not _NO_PASSES:
        _dedup_hoist_ldweights(nc)
    _spill_waits(nc)
    return nc


def _setup(ctx, tc, wa, w2p, b1, m128):
    nc = tc.nc

    const = ctx.enter_context(tc.tile_pool(name="const", bufs=1))
    ump = ctx.enter_context(tc.tile_pool(name="um", bufs=2))
    zp = ctx.enter_context(tc.tile_pool(name="z", bufs=3, space="PSUM"))
    up = ctx.enter_context(tc.tile_pool(name="u", bufs=2, space="PSUM"))

    wa_sb = const.tile([108, CH], _BF16)
    nc.sync.dma_start(out=wa_sb, in_=wa[:, :])
    w2p_sb = const.tile([CH + 1, 32], _BF16)
    nc.sync.dma_start(out=w2p_sb, in_=w2p[:, :])
    b1_sb = const.tile([CH, 1], _F32)
    nc.sync.dma_start(out=b1_sb, in_=b1[:, :])
    m128_sb = const.tile([128, MTILES * W], _BF16)
    nc.vector.memset(m128_sb[96:128, :], 0.0)
    nc.sync.dma_start(out=m128_sb[0:108, :], in_=m128[:, :])

    # manually double-buffered tiles (stable addresses):
    #  - ht: constant ones-row (row 96) carries b2 through the second matmul
    #  - xq: conv input, partitions (dj,di,c), straight from DRAM
    #  - xt: x in supertile layout (partitions 32j+c), holey 48-partition DMA
    #  - ot: output staging in supertile layout
    hts = [
        const.tile([CH + 1, ST, W], _BF16, name=f"ht{i}", tag=f"ht{i}")
        for i in range(2)
    ]
    xqs = [
        const.tile([108, XQLEN], _BF16, name=f"xqt{i}", tag=f"xqt{i}") for i in range(2)
    ]
    xts = [
        const.tile([128, ST, W], _BF16, name=f"xtt{i}", tag=f"xtt{i}") for i in range(2)
    ]
    ots = [
        const.tile([128, ST, W], _BF16, name=f"ott{i}", tag=f"ott{i}") for i in range(2)
    ]
    for t in hts:
        nc.vector.memset(t, 0.0)
        nc.vector.memset(t[CH : CH + 1, :, :], 1.0)
    for t in xqs + xts + ots:
        nc.vector.memset(t, 0.0)

    # warmup matmuls: absorb the weight-load DMA waits on the PE clock
    zw = zp.tile([CH, 2, 512], _F32, tag="z2")
    nc.tensor.matmul(zw[:, 0, 0:1], wa_sb, xqs[0][:, 0:1], start=True, stop=True)
    uw = up.tile([128, 512], _F32, tag="u")
    nc.tensor.matmul(
        uw[0:32, 0:1], w2p_sb, hts[0][:, 0, 0:1], start=True, stop=True,
        tile_position=(0, 0),
    )

    return dict(
        ump=ump, zp=zp, up=up,
        wa_sb=wa_sb, w2p_sb=w2p_sb, b1_sb=b1_sb, m128_sb=m128_sb,
        hts=hts, xqs=xqs, xts=xts, ots=ots,
    )


def _loop_body(tc, state, xrep, xjc, out):
    nc = tc.nc
    add = mybir.AluOpType.add
    mult = mybir.AluOpType.mult
    amax = mybir.AluOpType.max
    relu = mybir.ActivationFunctionType.Relu
    ump, zp, up = state["ump"], state["zp"], state["up"]
    wa_sb, w2p_sb, b1_sb, m128_sb = (
        state["wa_sb"], state["w2p_sb"], state["b1_sb"], state["m128_sb"],
    )
    hts, xqs, xts, ots = state["hts"], state["xqs"], state["xts"], state["ots"]

    nbuf = 0
    ncbuf = 0
    for b in range(BLOC):
        for chk in range(NCHUNK):
            r0 = chk * CHUNK
            xq = xqs[ncbuf % 2]
            xt = xts[ncbuf % 2]
            ot = ots[ncbuf % 2]
            ncbuf += 1

            # one DMA: partitions (di*12+c)*3 + dj <- row-and-column-shifted
            # copies of x rows r0..r0+15 (wrap-padded), straight from DRAM.
            # (di,c) outermost so the descriptor splitter spreads the 36
            # partition groups across all 16 SDMA engines, not 3.
            src = bass.AP(
                tensor=xrep,
                offset=(b * 3 * C * PADH + r0) * W2,
                ap=[[PADH * W2, 3 * C], [1, 3], [1, XQLEN]],
            )
            nc.sync.dma_start(out=xq, in_=src)
            # x in supertile layout: partitions 32j+c (pads host-zeroed)
            nc.sync.dma_start(out=xt[0:108, :, :], in_=xjc[b, chk, :, :])

            for st in range(NST):
                tglob = chk * NST + st
                tsim = (b * NCHUNK + chk) * NST + st
                z2a = zp.tile([CH, 2, 512], _F32, tag="z2")
                z2b = zp.tile([CH, 2, 512], _F32, tag="z2")
                # manual sim-time stamps group the conv and mm2 waves so the
                # PE stream stays weight-stationary within each wave; mm2 of
                # supertile t lands between conv(t+1) and conv(t+2)
                with tc.tile_wait_until(4 * tsim, enable=not _NO_STAMPS):
                    for j in range(ST):
                        q = st * ST + j
                        zt = (z2a if j < 2 else z2b)[:, j % 2, 0:W]
                        nc.tensor.matmul(
                            zt,
                            wa_sb,
                            xq[0:108, q * W2 : q * W2 + W],
                            start=True,
                            stop=True,
                        )

                ht = hts[nbuf % 2]
                nc.scalar.activation(
                    out=ht[0:CH, 0:2, :], in_=z2a[:, :, 0:W], func=relu, bias=b1_sb
                )
                nc.scalar.activation(
                    out=ht[0:CH, 2:3, :], in_=z2b[:, 0:1, 0:W], func=relu, bias=b1_sb
                )
                nc.vector.tensor_scalar(
                    ht[0:CH, 3:4, :], z2b[:, 1:2, 0:W], b1_sb, 0.0, add, amax
                )

                u = up.tile([128, 512], _F32, tag="u")
                with tc.tile_wait_until(4 * tsim + 6, enable=not _NO_STAMPS):
                    for j in range(ST):
                        nc.tensor.matmul(
                            u[32 * j : 32 * j + 32, 0:W],
                            w2p_sb,
                            ht[:, j, :],
                            start=True,
                            stop=True,
                            tile_position=(0, 32 * j),
                        )

                if b == 0 and chk == 0 and st == 0 and "dbg" in state:
                    dxq, dht, dxt = state["dbg"]
                    nc.sync.dma_start(out=dxq[:, :], in_=xq)
                    nc.sync.dma_start(out=dht[:, :, :], in_=ht)
                    nc.sync.dma_start(out=dxt[:, :, :], in_=xt)
                um = ump.tile([128, W], _BF16)
                nc.vector.tensor_tensor(
                    um, u[:, 0:W], m128_sb[:, tglob * W : tglob * W + W], mult
                )
                nc.gpsimd.tensor_tensor(
                    ot[:, st, :], um, xt[:, st, :], add
                )
                nbuf += 1

            nc.sync.dma_start(out=out[b, chk, :, :], in_=ot[0:108, :, :])


_NC_CACHE = {}


def _get_nc():
    if "nc" not in _NC_CACHE:
        _NC_CACHE["nc"] = _build_nc()
    return _NC_CACHE["nc"]


def _prep_inputs(x, w_perc, w1, b1, w2, b2, mask):
    bf16 = ml_dtypes.bfloat16
    wc = np.einsum("hp,pcij->hcij", w1, w_perc).astype(np.float32)  # [96,12,3,3]
    # wa[(di*12 + c)*3 + dj, h] = wc[h, c, di, dj]
    wdicdj = wc.transpose(2, 1, 3, 0)  # [di, c, dj, h]
    wa = np.ascontiguousarray(wdicdj.reshape(108, CH)).astype(bf16)
    w2pv = np.zeros((CH + 1, 32), np.float32)
    w2pv[0:CH, 0:C] = w2.T
    w2pv[CH, 0:C] = b2
    w2pv = w2pv.astype(bf16)
    b1c = np.ascontiguousarray(b1.reshape(CH, 1)).astype(np.float32)

    mbit = (mask > 0.5).astype(np.float32)
    m128 = np.zeros((108, MTILES * W), np.float32)
    for j in range(ST):
        rows = mbit[j::ST, :].reshape(MTILES * W)
        for c in range(C):
            m128[32 * j + c] = rows
    m128 = m128.astype(bf16)

    xb16 = x.astype(bf16)
    in_maps = []
    for core in range(NCORES):
        xs = xb16[core * BLOC : (core + 1) * BLOC]
        # di-replicated wrap-padded copy: xrep[b, di, c, r, w] = xpadbig[b, c, r+di, w]
        xpadbig = np.pad(xs, ((0, 0), (0, 0), (1, 5), (1, 1)), mode="wrap")
        xrep = np.stack(
            [xpadbig[:, :, di : di + PADH, :] for di in range(3)], axis=1
        )  # [BLOC, 3, C, PADH, W2]
        # supertile layout: xjc[b, chk, 32*j+c, s*W+w] = x[b, c, 16*chk+4*s+j, w]
        xs = xb16[core * BLOC : (core + 1) * BLOC].astype(np.float32)
        xst = np.zeros((BLOC, NCHUNK, ST, 32, NST, W), np.float32)
        xst[:, :, :, 0:C] = xs.reshape(BLOC, C, NCHUNK, NST, ST, W).transpose(
            0, 2, 4, 1, 3, 5
        )
        xjc = np.ascontiguousarray(
            xst.reshape(BLOC, NCHUNK, 128, STW)[:, :, 0:108].astype(ml_dtypes.bfloat16)
        )
        in_maps.append(
            {
                "xrep": np.ascontiguousarray(xrep),
                "xjc": xjc,
                "wa": wa,
                "w2p": w2pv,
                "b1": b1c,
                "m128": m128,
            }
        )
    return in_maps


def _unshard_out(core_outs):
    full = np.empty((B, C, H, W), np.float32)
    for core, o in enumerate(core_outs):
        o = np.asarray(o).astype(np.float32)
        o128 = np.zeros((BLOC, NCHUNK, 128, STW), np.float32)
        o128[:, :, 0:108] = o.reshape(BLOC, NCHUNK, 108, STW)
        o = o128.reshape(BLOC, NCHUNK, ST, 32, NST, W)[:, :, :, 0:C]
        # [b, chk, j, c, s, w] -> [b, c, (chk s j), w]
        o = o.transpose(0, 3, 1, 4, 2, 5).reshape(BLOC, C, H, W)
        full[core * BLOC : (core + 1) * BLOC] = o
    return full


def kernel(x, w_perc, w1, b1, w2, b2, mask):
    x = np.asarray(x, dtype=np.float32)
    in_maps = _prep_inputs(
        x,
        np.asarray(w_perc, np.float32),
        np.asarray(w1, np.float32),
        np.asarray(b1, np.float32),
        np.asarray(w2, np.float32),
        np.asarray(b2, np.float32),
        np.asarray(mask, np.float32),
    )
    nc = _get_nc()
    res = run_bass_kernel_spmd(nc, in_maps, core_ids=list(range(NCORES)))
    return _unshard_out([r["out"] for r in res.results])
